# revision 1
# baseline (speedup 1.0000x reference)
"""Trainium2 Bass kernel for AttentionalPlanarRemapping.

out[n,c,h,w] = sum_d softmax(atts[n,c,:])[d] * images[n,d,h,w]

Per-sample: W = softmax(atts[n]) [C,C]; out[n] = W @ images[n].reshape(C, H*W).

Sharding: data-parallel over N across 8 cores (4 samples per core).

Host preprocessing inside kernel(): atts is passed TRANSPOSED per sample
(attsT[n] = atts[n].T, layout [d, c]) so no on-device transposition of the
512x512 weight matrix is needed: attsT loads with the contraction dim d on
partitions, which is exactly the matmul lhsT layout.

images are uploaded as fp16 and the output is stored as fp16 (values only --
the returned array is float32): fp16's 11-bit mantissa matches the PE's
reduced-precision matmul path while halving DMA traffic, which is the
bottleneck. The softmax logits/denominators stay f32/f32r.

Per-core plan (software-pipelined: prep(n+1) is emitted before compute(n)
so the next sample's loads/exp are never queued behind this sample's
evictions):
  prep(n):
    1. DMA attsT[n] (2 halves) -> A [128, 2(kd), 512(c)] f32 (d on parts)
    2. DMA images[n] (2 halves) -> X [128, 2(kd), 1024] fp16
    3. E = exp(A) -> fp16 (ACT; no max-sub: |atts| < 6 so exp is safe)
  compute(n):
    4. s_ps[p,c] = sum_d E[d,c] replicated across partitions (ones.T @ E, PE)
    5. s_sb = f32r copy of s_ps (DVE)
    6. main matmuls on UNNORMALIZED E: psum[c128, hw1024] += E-blk.T @ X-blk
    7. after kc=0's matmuls: redistribute s to per-partition layout via tiny
       PE matmuls (s_sb-blk.T @ (1/128)) -> [128, KC]; r = 1/s (DVE, tiny)
    8. evict psum -> O fp16 with per-partition scale r, alternating ACT/DVE
    9. DMA O -> out[n] (2 halves: SWDGE + scalar-HWDGE, so stores do not
       block the sync load queue)
"""

import numpy as np
from contextlib import ExitStack

import concourse.bass as bass
import concourse.mybir as mybir
import concourse.tile as tile
from concourse import bacc
from concourse.bass_utils import run_bass_kernel_spmd

N, C, H, W = 32, 512, 32, 32
HW = H * W                      # 1024
NCORES = 8
NPC = N // NCORES               # 4 samples per core
P = 128
KC = C // P                     # 4 chunks over output channel c
KD = C // P                     # 4 chunks over contraction d
NT = 512                        # matmul moving free dim (one PSUM bank of f32)
NHT = HW // NT                  # 2

F32 = mybir.dt.float32
F32R = mybir.dt.float32r
F16 = mybir.dt.float16
AF = mybir.ActivationFunctionType
AX = mybir.AxisListType


def build_nc():
    nc = bacc.Bacc("TRN2", target_bir_lowering=False, debug=False)

    images = nc.dram_tensor("images", [NPC, C, HW], F16, kind="ExternalInput").ap()
    attsT = nc.dram_tensor("attsT", [NPC, C, C], F32, kind="ExternalInput").ap()
    out = nc.dram_tensor("out", [NPC, C, HW], F16, kind="ExternalOutput").ap()

    with ExitStack() as ctx:
        tc = ctx.enter_context(tile.TileContext(nc))

        const_pool = ctx.enter_context(tc.tile_pool(name="const", bufs=1))
        ones_f32 = const_pool.tile([P, P], F32)
        ones = const_pool.tile([P, P], F16)
        oinv_f32 = const_pool.tile([P, 2], F32)
        oinv = const_pool.tile([P, 2], F32R)

        a_pool = ctx.enter_context(tc.tile_pool(name="a", bufs=2))
        e_pool = ctx.enter_context(tc.tile_pool(name="e", bufs=2))
        x_pool = ctx.enter_context(tc.tile_pool(name="x", bufs=3))
        o_pool = ctx.enter_context(tc.tile_pool(name="o", bufs=NPC))
        st_pool = ctx.enter_context(tc.tile_pool(name="st", bufs=2))
        sm_psum = ctx.enter_context(tc.tile_pool(name="smp", bufs=1, space="PSUM"))
        mm_psum = ctx.enter_context(tc.tile_pool(name="mmp", bufs=3, space="PSUM"))


        def prep(n, first):
            """Input DMAs + exp for sample n (one sample ahead of compute)."""
            a_h = []
            for h in range(2):
                a_t = a_pool.tile([P, 2, C], F32, name=f"a{n}_{h}", tag=f"a{h}")
                nc.sync.dma_start(
                    a_t[:],
                    attsT[n][h * 256 : (h + 1) * 256].rearrange(
                        "(kd p) c -> p kd c", p=P
                    ),
                )
                a_h.append(a_t)
            x_h = []
            for h in range(2):
                x_t = x_pool.tile([P, 2, HW], F16, name=f"x{n}_{h}", tag=f"x{h}")
                nc.sync.dma_start(
                    x_t[:],
                    images[n][h * 256 : (h + 1) * 256].rearrange(
                        "(kd p) f -> p kd f", p=P
                    ),
                )
                x_h.append(x_t)

            if first:
                nc.gpsimd.memset(ones_f32[:], 1.0)
                nc.vector.tensor_copy(ones[:], ones_f32[:])
                nc.gpsimd.memset(oinv_f32[:], 1.0 / P)
                nc.vector.tensor_copy(oinv[:], oinv_f32[:])

            # E = exp(attsT) in [d, c] layout; no max-sub (|atts| < 6)
            e_t = e_pool.tile([P, KD, C], F16, name=f"e{n}", tag="e")
            for h in range(2):
                nc.scalar.activation(
                    e_t[:, h * 2 : (h + 1) * 2],
                    a_h[h][:],
                    AF.Exp,
                    bias=0.0,
                    scale=1.0,
                )
            return e_t, x_h

        def compute(n, e_t, x_h):
            # denominators replicated: R[p, c] = sum_d E[d, c]
            s_ps = sm_psum.tile([P, C], F32, name=f"s{n}", tag="s", space="PSUM")
            for kd in range(KD):
                nc.tensor.matmul(
                    s_ps[:],
                    lhsT=ones[:],
                    rhs=e_t[:, kd],
                    start=(kd == 0),
                    stop=(kd == KD - 1),
                )
            s_sb = st_pool.tile([P, C], F32R, name=f"ssb{n}", tag="ssb")
            nc.vector.tensor_copy(s_sb[:], s_ps[:])

            # main matmuls on UNNORMALIZED E; normalize at eviction. The r
            # redistribution (tiny PE matmuls) is emitted after kc=0's
            # matmuls so the DVE reciprocal completes without a PE stall.
            r_sb = st_pool.tile([P, KC], F32, name=f"rsb{n}", tag="rsb")
            for kc in range(KC):
                ps = mm_psum.tile(
                    [P, HW], F32, name=f"ps{n}_{kc}", tag="ps", space="PSUM"
                )
                for kd in range(KD):
                    for ht in range(NHT):
                        nc.tensor.matmul(
                            ps[:, ht * NT : (ht + 1) * NT],
                            lhsT=e_t[:, kd, kc * P : (kc + 1) * P],
                            rhs=x_h[kd // 2][:, kd % 2, ht * NT : (ht + 1) * NT],
                            start=(kd == 0),
                            stop=(kd == KD - 1),
                        )
                if kc == 0:
                    # rp[c_p, j] = sum_k s_sb[k, c-blk] * (1/128) = s[c]  (PE)
                    rp_ps = sm_psum.tile(
                        [P, 2 * KC], F32, name=f"rp{n}", tag="rp", space="PSUM"
                    )
                    for j in range(KC):
                        nc.tensor.matmul(
                            rp_ps[:, j * 2 : (j + 1) * 2],
                            lhsT=s_sb[:, j * P : (j + 1) * P],
                            rhs=oinv[:],
                        )
                    s_col = st_pool.tile([P, KC], F32, name=f"scol{n}", tag="scol")
                    nc.vector.tensor_copy(
                        s_col[:],
                        rp_ps[:].rearrange("p (kc j) -> p kc j", j=2)[:, :, 0],
                    )
                    nc.vector.reciprocal(r_sb[:], s_col[:])
                # per-kc eviction + immediate store: output streams to HBM
                # as soon as each 128-row band is normalized, instead of
                # waiting for 1MB halves to fill
                o_t = o_pool.tile([P, HW], F16, name=f"o{n}_{kc}", tag=f"o{kc}")
                r_ap = r_sb[:, kc : kc + 1]
                if kc % 2 == 0:
                    nc.scalar.mul(o_t[:], ps[:], r_ap)
                    nc.gpsimd.dma_start(out[n][kc * P : (kc + 1) * P], o_t[:])
                else:
                    nc.vector.tensor_scalar_mul(o_t[:], ps[:], r_ap)
                    nc.scalar.dma_start(out[n][kc * P : (kc + 1) * P], o_t[:])

        # software pipeline: prep one sample ahead so the next sample's
        # exp/loads are never queued behind this sample's evictions
        staged = prep(0, True)
        for n in range(NPC):
            nxt = prep(n + 1, False) if n + 1 < NPC else None
            compute(n, *staged)
            staged = nxt


    nc.compile()
    return nc


_NC_CACHE = None


def _get_nc():
    global _NC_CACHE
    if _NC_CACHE is None:
        _NC_CACHE = build_nc()
    return _NC_CACHE


def run(in_maps, **kwargs):
    """Run the SPMD kernel on cores 0..7. in_maps: one dict per core."""
    nc = _get_nc()
    return run_bass_kernel_spmd(nc, in_maps, core_ids=list(range(NCORES)), **kwargs)


def make_in_maps(images: np.ndarray, atts: np.ndarray):
    images = np.ascontiguousarray(np.asarray(images, dtype=np.float32).astype(np.float16))
    atts = np.asarray(atts, dtype=np.float32)
    assert images.shape == (N, C, H, W), images.shape
    assert atts.shape == (N, C, C), atts.shape
    img_s = images.reshape(NCORES, NPC, C, HW)
    # per-sample transpose: attsT[n] = atts[n].T  (layout [d, c])
    attsT = np.ascontiguousarray(atts.transpose(0, 2, 1)).reshape(
        NCORES, NPC, C, C
    )
    return [
        {"images": np.ascontiguousarray(img_s[i]), "attsT": attsT[i]}
        for i in range(NCORES)
    ]


def kernel(images: np.ndarray, atts: np.ndarray) -> np.ndarray:
    in_maps = make_in_maps(images, atts)
    res = run(in_maps)
    outs = [res.results[i]["out"] for i in range(NCORES)]
    full = np.concatenate(outs, axis=0).reshape(N, C, H, W)
    return full.astype(np.float32)

